# revision 136
# baseline (speedup 1.0000x reference)
"""GQA attention block (RMSNorm + QKV proj + partial RoPE + causal GQA
attention + XSA correction + out proj) on 8 trn2 NeuronCores.

Sharding: 2 batches x 4 KV-groups (each core: 1 batch, 1 kv head, 4 q heads).
Each core computes a partial output (its 4 heads through its wo column slice);
the host sums the 4 partials per batch.

Layout strategy (v2, ~2.07x faster than v1 in TimelineSim):
- x / wqkv shipped bf16 (halves input DMA); QKV + scores matmuls in bf16,
  PV and out-proj bf16, PSUM accumulation f32 throughout.
- PSUM budget: tag Q (3 banks: QKV accumulation in two 3-output passes,
  then ps_pv per head), tag B (3 banks: scores / v-transpose / out-proj),
  tag S (2 banks: rmsnorm row-sum + softmax denominators).  Phases
  pipeline instead of serializing on one 6-buffer pool.
- out-projection of chunk j-1 emitted between QKV(j) and attention(j):
  PE filler that hides the rope/evac/v-transpose dependency chains.
- causal masking: diagonal blocks use partial-width matmuls (only the
  unmasked query range) + one 128x128 triangle mask multiply on DVE.
- softmax denominator via ones-vector matmul; XSA dot/vns terms via
  gpsimd.partition_all_reduce (no PE or PSUM cost); the XSA correction
  is v-scale-invariant so vns uses the rms-scaled v directly.
- x loads batched 4 D-chunks per DMA descriptor set; stores bf16 with
  f32 summation of the 4 kv-group partials on host.
- emission order tuned so the in-order DVE/Pool/ACT queues resolve each
  PE dependency before PE reaches it: QKV pass A computes (q0, k, v) so
  the k-rope / v-scale / v-transpose chains drain under pass B's matmuls;
  rs accumulation shifted 6 rounds; v-transposes emitted inside the
  out-projection block; Exp act-table preloaded at startup.
"""

import sys

for _p in ("/opt/trn_rl_repo", "/root/.axon_site/_ro/trn_rl_repo"):
    if _p not in sys.path:
        sys.path.append(_p)

import numpy as np
import ml_dtypes

import concourse.bass as bass
import concourse.bacc as bacc
import concourse.mybir as mybir
import concourse.tile as tile
from concourse.bass_utils import run_bass_kernel_spmd
from concourse.masks import make_identity

F32 = mybir.dt.float32
F32R = mybir.dt.float32r
BF16 = mybir.dt.bfloat16

B, T, D = 2, 2048, 2048
NH, NKV, HD = 16, 4, 128
RD = 64  # rope dims
NH_L = NH // NKV           # 4 q heads per core
EL = (NH_L + 2) * HD       # 768: q0..q3, k, v
TC = 512                   # token chunk
NTC = T // TC              # 4
DC = D // 128              # 16 contraction chunks
S128 = float(1.0 / np.sqrt(HD))
EPS = 1e-6

_CACHE = {}


def _build_nc():
    nc = bacc.Bacc("TRN2", target_bir_lowering=False, debug=False)

    xT = nc.declare_dram_parameter("xT", [D, T], BF16, isOutput=False)
    wT = nc.declare_dram_parameter("wqkvT", [D, EL], BF16, isOutput=False)
    woT = nc.declare_dram_parameter("woT", [NH_L * HD, D], BF16, isOutput=False)
    csP = nc.declare_dram_parameter("cs", [128, T], F32, isOutput=False)
    outp = nc.declare_dram_parameter("out", [T, D], BF16, isOutput=True)

    ACT = mybir.ActivationFunctionType

    with tile.TileContext(nc) as tc:
        with (
            nc.allow_low_precision(reason="bf16/fp32r matmuls; tolerances ok"),
            tc.tile_pool(name="singles", bufs=1) as sg,
            tc.tile_pool(name="stream", bufs=2) as st,
            tc.tile_pool(name="ps", bufs=1, space="PSUM") as ps,
        ):
            # ---- persistent tiles ----
            w_t = [sg.tile([128, EL], BF16, tag=f"w{i}", name=f"w{i}")
                   for i in range(DC)]
            wo_sb = [sg.tile([128, D], BF16, tag=f"wo{h}", name=f"wo{h}")
                     for h in range(NH_L)]
            cos_sb = sg.tile([RD, T], F32, tag="cos")
            sin_sb = sg.tile([RD, T], F32, tag="sin")
            ident = sg.tile([128, 128], BF16, tag="ident")
            ones_cb = sg.tile([128, 1], BF16, tag="ones_cb")
            eps_t = sg.tile([1, 1], F32, tag="eps_t")
            tri = sg.tile([128, 128], BF16, tag="tri")
            qhat = [
                [sg.tile([128, TC], BF16, tag=f"qh{h}_{j}", name=f"qh{h}_{j}")
                 for j in range(NTC)]
                for h in range(NH_L)
            ]
            khat = [sg.tile([128, TC], BF16, tag=f"kh{j}", name=f"kh{j}")
                    for j in range(NTC)]
            vhat = [sg.tile([128, TC], BF16, tag=f"vh{j}", name=f"vh{j}")
                    for j in range(NTC)]
            vtok = [sg.tile([128, TC], BF16, tag=f"vt{j}", name=f"vt{j}")
                    for j in range(NTC)]
            aout = [
                [sg.tile([128, TC], BF16, tag=f"ao{h}_{j}", name=f"ao{h}_{j}")
                 for j in range(NTC)]
                for h in range(NH_L)
            ]
            vhat2 = [sg.tile([128, TC], BF16, tag=f"vh2_{j}", name=f"vh2_{j}")
                     for j in range(NTC)]

            xts = {}

            def prefetch_x(j, g, split=1):
                """One DMA for D-chunks 4g..4g+3 of token chunk j."""
                tl = st.tile([128, 4 * TC], BF16, tag="xt", bufs=8,
                             name=f"xt{j}_{g}")
                xts[(j, g)] = tl
                nch = 4 // split
                for s in range(split):
                    src = xT[g * 512 + s * nch * 128:
                             g * 512 + (s + 1) * nch * 128,
                             j * TC:(j + 1) * TC]
                    if nch == 1:
                        nc.sync.dma_start(
                            out=tl[:, s * TC:(s + 1) * TC], in_=src)
                    else:
                        nc.sync.dma_start(
                            out=tl[:, s * nch * TC:(s + 1) * nch * TC]
                            .rearrange("p (c t) -> p c t", c=nch),
                            in_=src.rearrange("(c p) t -> p c t", p=128),
                        )

            def xt_slice(j, i):
                return xts[(j, i // 4)][:, (i % 4) * TC:(i % 4 + 1) * TC]

            # cs rides the ACT queue so w/x start immediately on SP
            nc.scalar.dma_start(out=cos_sb, in_=csP[0:RD, :])
            nc.scalar.dma_start(out=sin_sb, in_=csP[RD:128, :])
            # interleave w/x loads so QKV(0) can start after the first pair
            for g in range(4):
                prefetch_x(0, g, split=(2 if g == 0 else 1))
                for i in range(4 * g, 4 * g + 4):
                    nc.sync.dma_start(out=w_t[i],
                                      in_=wT[i * 128:(i + 1) * 128, :])

            # constants generated while the startup DMAs stream
            make_identity(nc, ident)
            nc.vector.memset(ones_cb, 1.0)
            nc.vector.memset(eps_t, EPS)
            # dummy exp: pulls the Exp act-table load into the startup
            # DMA wait instead of the first attention tile
            warm = st.tile([1, 1], F32, tag="warm", bufs=1)
            nc.scalar.activation(warm, eps_t, ACT.Exp, scale=1.0)
            nc.gpsimd.memset(tri, 1.0)
            # tri[k, q] = 1 where q >= k else 0 (diagonal 128x128 block)
            nc.gpsimd.affine_select(
                out=tri, in_=tri,
                compare_op=mybir.AluOpType.is_ge,
                fill=0.0,
                base=0,
                pattern=[[1, 128]],
                channel_multiplier=-1,
            )

            def emit_out_proj(jm1, mid_cb=None):
                """Out-projection for token chunk jm1 (PE filler work)."""
                for tl in range(4):
                    tt = jm1 * 4 + tl
                    if tl == 1 and mid_cb is not None:
                        mid_cb()
                    osb = st.tile([128, D], BF16, tag="osb", bufs=2,
                                  name=f"osb{tt}")
                    # final tile of the run: store per-m so the drain only
                    # waits on the last 512-column transfer
                    last = (jm1 == NTC - 1 and tl == 3)
                    for m in range(4):
                        ps_o = ps.tile([128, TC], F32, tag="B", bufs=3)
                        for h2 in range(NH_L):
                            nc.tensor.matmul(
                                ps_o,
                                aout[h2][jm1][:, tl * 128:(tl + 1) * 128],
                                wo_sb[h2][:, m * TC:(m + 1) * TC],
                                start=(h2 == 0), stop=(h2 == NH_L - 1),
                            )
                        if m % 2 == 0 or tl == 3:
                            nc.vector.tensor_copy(
                                osb[:, m * TC:(m + 1) * TC], ps_o)
                        else:
                            nc.scalar.copy(
                                osb[:, m * TC:(m + 1) * TC], ps_o)
                        if last:
                            nc.sync.dma_start(
                                out=outp[tt * 128:(tt + 1) * 128,
                                         m * TC:(m + 1) * TC],
                                in_=osb[:, m * TC:(m + 1) * TC])
                    if not last:
                        nc.sync.dma_start(
                            out=outp[tt * 128:(tt + 1) * 128, :], in_=osb)

            for j in range(NTC):
                js = slice(j * TC, (j + 1) * TC)
                # -------------- QKV pass A: q0, k, v + rs ------------------
                # k/v in pass A so their rope / v-scale / transpose chains
                # on DVE+Pool resolve while pass B runs on PE
                ps_rs = ps.tile([1, TC], F32, tag="S", bufs=2)
                psA = [ps.tile([128, TC], F32, tag="Q", bufs=3,
                               name=f"psA{j}_{e}") for e in range(3)]
                def emit_rs(i):
                    x2 = st.tile([128, TC], BF16, tag="x2", bufs=2)
                    nc.vector.tensor_mul(x2, xt_slice(j, i), xt_slice(j, i))
                    nc.tensor.matmul(ps_rs, ones_cb, x2,
                                     start=(i == 0), stop=(i == DC - 1))

                # rs matmuls shifted 6 behind qkv so chunk j's first PE inst
                # doesn't stall on the S-bank freed by attention(j-1) tail
                for i in range(DC):
                    xt_ji = xt_slice(j, i)
                    for ei, e in enumerate((0, 4, 5)):   # q0, k, v
                        nc.tensor.matmul(
                            psA[ei], w_t[i][:, e * 128:(e + 1) * 128], xt_ji,
                            start=(i == 0), stop=(i == DC - 1),
                        )
                    if i >= 6:
                        emit_rs(i - 6)
                for i in range(DC - 6, DC):
                    emit_rs(i)
                # rs = 1/sqrt(mean(x^2)+eps), broadcast across partitions
                sq = st.tile([1, TC], F32, tag="sq", bufs=1)
                nc.scalar.activation(sq, ps_rs, ACT.Sqrt, scale=1.0 / D,
                                     bias=eps_t)
                rs_t = st.tile([1, TC], F32, tag="rs_t", bufs=1)
                nc.vector.reciprocal(rs_t, sq)
                rsb = st.tile([128, TC], F32, tag="rsb", bufs=2)
                nc.gpsimd.partition_broadcast(rsb, rs_t)
                # fold rs into rope tables (in place, this chunk's columns)
                nc.vector.tensor_mul(cos_sb[:, js], cos_sb[:, js], rsb[0:RD])
                nc.vector.tensor_mul(sin_sb[:, js], sin_sb[:, js], rsb[0:RD])

                def rope(th, fast=False):
                    # fast=True keeps the whole chain on DVE: used for the
                    # tensors gating the first score matmuls (k, q0)
                    dt = th.dtype
                    t2s = st.tile([RD, TC], dt, tag="t2s", bufs=6)
                    t1 = st.tile([RD, TC], dt, tag="t1", bufs=3)
                    qeng = nc.scalar if fast else nc.sync
                    qeng.dma_start(out=t2s[0:32], in_=th[32:64])
                    qeng.dma_start(out=t2s[32:64], in_=th[0:32])
                    mul2 = nc.vector if fast else nc.gpsimd
                    mul2.tensor_mul(t2s, t2s, sin_sb[:, js])
                    nc.vector.tensor_mul(t1, th[0:RD], cos_sb[:, js])
                    nc.vector.tensor_add(th[0:RD], t1, t2s)
                    mul2.tensor_mul(th[RD:128], th[RD:128], rsb[RD:128])

                # evac pass A; k/q0 rope + vhat chains run under pass B
                nc.scalar.copy(khat[j], psA[1])
                nc.scalar.copy(vhat[j], psA[2])
                nc.vector.tensor_mul(vhat[j], vhat[j], rsb)
                nc.scalar.copy(qhat[0][j], psA[0])
                rope(khat[j], fast=(j == 0))
                rope(qhat[0][j], fast=(j == 0))

                # ---------------- QKV pass B: q1,q2,q3 ---------------------
                psB = [ps.tile([128, TC], F32, tag="Q", bufs=3,
                               name=f"psB{j}_{e}") for e in range(3)]
                for i in range(DC):
                    for e in range(3):
                        nc.tensor.matmul(
                            psB[e],
                            w_t[i][:, (1 + e) * 128:(2 + e) * 128],
                            xt_slice(j, i),
                            start=(i == 0), stop=(i == DC - 1),
                        )
                for h in range(1, 4):
                    nc.scalar.copy(qhat[h][j], psB[h - 1])
                    rope(qhat[h][j])

                # XSA v/|v|^2 term: only the head epilogues need it
                vsq = st.tile([128, TC], BF16, tag="vsq", bufs=1)
                nc.gpsimd.tensor_mul(vsq, vhat[j], vhat[j])
                vns_b = st.tile([128, TC], F32, tag="vns_b", bufs=1)
                nc.gpsimd.partition_all_reduce(
                    vns_b, vsq, channels=128,
                    reduce_op=bass.bass_isa.ReduceOp.add,
                )
                rvnsb = st.tile([128, TC], F32, tag="rvnsb", bufs=1)
                nc.vector.reciprocal(rvnsb, vns_b)
                nc.vector.tensor_mul(vhat2[j], vhat[j], rvnsb)

                def emit_vt():
                    ps_vt = ps.tile([128, TC], BF16, tag="B", bufs=3)
                    for kk in range(TC // 128):
                        nc.tensor.transpose(
                            ps_vt[:, kk * 128:(kk + 1) * 128],
                            vhat[j][:, kk * 128:(kk + 1) * 128],
                            ident,
                        )
                    if j == 0:
                        nc.vector.tensor_copy(vtok[j], ps_vt)
                    else:
                        nc.scalar.copy(vtok[j], ps_vt)

                # ---------------- attention for this q chunk ---------------
                nkt = 4 * (j + 1)
                ps_pv = {}
                ps_sum = {}
                pts = {}
                prefilled = set()

                def emit_sc(h, kt):
                    # diagonal blocks: queries < kt*128 are masked anyway,
                    # so compute only columns [lo:] (partial-width matmul)
                    lo = (kt - 4 * j) * 128 if kt >= 4 * j else 0
                    ps_sc = ps.tile([128, TC], F32, tag="B", bufs=3)
                    nc.tensor.matmul(
                        ps_sc[:, lo:],
                        khat[kt // 4][:, (kt % 4) * 128:(kt % 4 + 1) * 128],
                        qhat[h][j][:, lo:],
                        start=True, stop=True,
                    )
                    pT = st.tile([128, TC], BF16, tag="pT", bufs=6)
                    nc.scalar.activation(pT[:, lo:], ps_sc[:, lo:],
                                         ACT.Exp, scale=S128)
                    if kt >= 4 * j:  # causal triangle on first 128 cols
                        nc.vector.tensor_mul(
                            pT[:, lo:lo + 128], pT[:, lo:lo + 128], tri)
                    pts[(h, kt)] = (pT, lo)

                def emit_acc(h, kt):
                    pT, lo = pts.pop((h, kt))
                    nc.tensor.matmul(
                        ps_pv[h][:, lo:],
                        vtok[kt // 4][:, (kt % 4) * 128:(kt % 4 + 1) * 128],
                        pT[:, lo:],
                        start=(kt == 0), stop=(kt == nkt - 1),
                    )
                    nc.tensor.matmul(
                        ps_sum[h][:, lo:], ones_cb, pT[:, lo:],
                        start=(kt == 0), stop=(kt == nkt - 1),
                    )

                def emit_epilogue(h):
                    # normalization + XSA correction:
                    # aout = (pv - vhat2 * allreduce(pv*vhat)) / rowsum
                    # tu chain first: it is the longer serial path
                    tu = st.tile([128, TC], F32R, tag="tu", bufs=2)
                    nc.vector.tensor_mul(tu, ps_pv[h], vhat[j])
                    dotb = st.tile([128, TC], F32, tag="dotb", bufs=3)
                    nc.gpsimd.partition_all_reduce(
                        dotb, tu, channels=128,
                        reduce_op=bass.bass_isa.ReduceOp.add,
                    )
                    inv = st.tile([1, TC], F32, tag="inv", bufs=2)
                    nc.vector.reciprocal(inv, ps_sum[h])
                    ibc = st.tile([128, TC], F32, tag="bc", bufs=3, name="ibc")
                    nc.gpsimd.partition_broadcast(ibc, inv)
                    m2 = st.tile([128, TC], F32, tag="m2", bufs=3)
                    # j=0: kt loops are too thin to drain an all-DVE chain
                    eng2 = nc.gpsimd if j == 0 else nc.vector
                    eng2.tensor_mul(m2, vhat2[j], dotb)
                    s_ = st.tile([128, TC], F32, tag="s_", bufs=3)
                    nc.vector.tensor_sub(s_, ps_pv[h], m2)
                    eng2.tensor_mul(aout[h][j], s_, ibc)

                def prefill(h, depth):
                    ps_pv[h] = ps.tile([128, TC], F32, tag="Q", bufs=3,
                                       name=f"pv{j}_{h}")
                    ps_sum[h] = ps.tile([1, TC], F32, tag="S", bufs=2,
                                        name=f"sum{j}_{h}")
                    for kt in range(min(depth, nkt)):
                        emit_sc(h, kt)
                    prefilled.add(h)

                # -------- out-projection for the previous chunk ------------
                if j > 0:
                    # x prefetch first: no deps, so it issues on SP before
                    # the osb stores (whose sem-waits would block the queue)
                    if j + 1 < NTC:
                        for g in range(4):
                            prefetch_x(j + 1, g)
                    emit_out_proj(j - 1, mid_cb=emit_vt)
                else:
                    # fill the pre-transpose bubble with h0's first scores
                    prefill(0, 3)
                    emit_vt()
                    if NTC > 1:
                        for g in range(4):
                            prefetch_x(1, g)
                        # wo loads behind the x prefetch; needed from j=1
                        for h in range(NH_L):
                            nc.sync.dma_start(
                                out=wo_sb[h],
                                in_=woT[h * 128:(h + 1) * 128, :]
                            )

                for h in range(NH_L):
                    if h not in prefilled:
                        prefill(h, 3)
                    for kt in range(nkt):
                        if kt + 3 < nkt:
                            emit_sc(h, kt + 3)
                        emit_acc(h, kt)
                    emit_epilogue(h)

            # tail: out-projection of the last chunk
            emit_out_proj(NTC - 1)

    nc.compile()
    return nc


def _host_inputs(x, cos, sin, w_norm, wq, wk, wv, wo):
    """Build the 8 per-core input maps (host-side layout prep only)."""
    wn = w_norm.astype(np.float32)
    cosT = cos.T.astype(np.float32)                                # [64, T]
    sinT = sin.T.astype(np.float32)
    sinS = np.concatenate([-sinT[:32], sinT[32:]], axis=0)         # [64, T]
    cs = np.ascontiguousarray(np.concatenate([cosT, sinS], axis=0))  # [128, T]
    xTs = [np.ascontiguousarray(x[b].T).astype(ml_dtypes.bfloat16)
           for b in range(B)]
    in_maps = []
    for c in range(8):
        b, g = divmod(c, 4)
        wq_s = wq[g * NH_L * HD:(g + 1) * NH_L * HD] * wn[None, :]
        wk_s = wk[g * HD:(g + 1) * HD] * wn[None, :]
        wv_s = wv[g * HD:(g + 1) * HD] * wn[None, :]
        wqkvT = np.ascontiguousarray(
            np.concatenate([wq_s, wk_s, wv_s], axis=0).T
        ).astype(ml_dtypes.bfloat16)                               # [D, 768]
        woT_s = np.ascontiguousarray(
            wo[:, g * NH_L * HD:(g + 1) * NH_L * HD].T
        ).astype(ml_dtypes.bfloat16)                               # [512, D]
        in_maps.append({
            "xT": xTs[b],
            "wqkvT": wqkvT,
            "woT": woT_s,
            "cs": cs,
        })
    return in_maps


def kernel(x, cos, sin, w_norm, wq, wk, wv, wo, rope_dims=64, use_xsa=1,
           **_unused):
    if "nc" not in _CACHE:
        _CACHE["nc"] = _build_nc()
    nc = _CACHE["nc"]
    in_maps = _host_inputs(
        np.asarray(x), np.asarray(cos), np.asarray(sin), np.asarray(w_norm),
        np.asarray(wq), np.asarray(wk), np.asarray(wv), np.asarray(wo),
    )
    res_obj = run_bass_kernel_spmd(nc, in_maps, list(range(8)))
    _CACHE["last"] = res_obj
    res = res_obj.results
    out = np.zeros((B, T, D), dtype=np.float32)
    for c in range(8):
        b = c // 4
        out[b] += np.asarray(res[c]["out"], dtype=np.float32)
    return out


# revision 141
# speedup vs baseline: 1.0005x; 1.0005x over previous
"""GQA attention block (RMSNorm + QKV proj + partial RoPE + causal GQA
attention + XSA correction + out proj) on 8 trn2 NeuronCores.

Sharding: 2 batches x 4 KV-groups (each core: 1 batch, 1 kv head, 4 q heads).
Each core computes a partial output (its 4 heads through its wo column slice);
the host sums the 4 partials per batch.

Layout strategy (v2, ~2.07x faster than v1 in TimelineSim):
- x / wqkv shipped bf16 (halves input DMA); QKV + scores matmuls in bf16,
  PV and out-proj bf16, PSUM accumulation f32 throughout.
- PSUM budget: tag Q (3 banks: QKV accumulation in two 3-output passes,
  then ps_pv per head), tag B (3 banks: scores / v-transpose / out-proj),
  tag S (2 banks: rmsnorm row-sum + softmax denominators).  Phases
  pipeline instead of serializing on one 6-buffer pool.
- out-projection of chunk j-1 emitted between QKV(j) and attention(j):
  PE filler that hides the rope/evac/v-transpose dependency chains.
- causal masking: diagonal blocks use partial-width matmuls (only the
  unmasked query range) + one 128x128 triangle mask multiply on DVE.
- softmax denominator via ones-vector matmul; XSA dot/vns terms via
  gpsimd.partition_all_reduce (no PE or PSUM cost); the XSA correction
  is v-scale-invariant so vns uses the rms-scaled v directly.
- x loads batched 4 D-chunks per DMA descriptor set; stores bf16 with
  f32 summation of the 4 kv-group partials on host.
- emission order tuned so the in-order DVE/Pool/ACT queues resolve each
  PE dependency before PE reaches it: QKV pass A computes (q0, k, v) so
  the k-rope / v-scale / v-transpose chains drain under pass B's matmuls;
  rs accumulation shifted 6 rounds; v-transposes emitted inside the
  out-projection block; Exp act-table preloaded at startup.
"""

import sys

for _p in ("/opt/trn_rl_repo", "/root/.axon_site/_ro/trn_rl_repo"):
    if _p not in sys.path:
        sys.path.append(_p)

import numpy as np
import ml_dtypes

import concourse.bass as bass
import concourse.bacc as bacc
import concourse.mybir as mybir
import concourse.tile as tile
from concourse.bass_utils import run_bass_kernel_spmd
from concourse.masks import make_identity

F32 = mybir.dt.float32
F32R = mybir.dt.float32r
BF16 = mybir.dt.bfloat16

B, T, D = 2, 2048, 2048
NH, NKV, HD = 16, 4, 128
RD = 64  # rope dims
NH_L = NH // NKV           # 4 q heads per core
EL = (NH_L + 2) * HD       # 768: q0..q3, k, v
TC = 512                   # token chunk
NTC = T // TC              # 4
DC = D // 128              # 16 contraction chunks
S128 = float(1.0 / np.sqrt(HD))
EPS = 1e-6

_CACHE = {}


def _build_nc():
    nc = bacc.Bacc("TRN2", target_bir_lowering=False, debug=False)

    xT = nc.declare_dram_parameter("xT", [D, T], BF16, isOutput=False)
    wT = nc.declare_dram_parameter("wqkvT", [D, EL], BF16, isOutput=False)
    woT = nc.declare_dram_parameter("woT", [NH_L * HD, D], BF16, isOutput=False)
    csP = nc.declare_dram_parameter("cs", [128, T], F32, isOutput=False)
    outp = nc.declare_dram_parameter("out", [T, D], BF16, isOutput=True)

    ACT = mybir.ActivationFunctionType

    with tile.TileContext(nc) as tc:
        with (
            nc.allow_low_precision(reason="bf16/fp32r matmuls; tolerances ok"),
            tc.tile_pool(name="singles", bufs=1) as sg,
            tc.tile_pool(name="stream", bufs=2) as st,
            tc.tile_pool(name="ps", bufs=1, space="PSUM") as ps,
        ):
            # ---- persistent tiles ----
            w_t = [sg.tile([128, EL], BF16, tag=f"w{i}", name=f"w{i}")
                   for i in range(DC)]
            wo_sb = [sg.tile([128, D], BF16, tag=f"wo{h}", name=f"wo{h}")
                     for h in range(NH_L)]
            cos_sb = sg.tile([RD, T], F32, tag="cos")
            sin_sb = sg.tile([RD, T], F32, tag="sin")
            ident = sg.tile([128, 128], BF16, tag="ident")
            ones_cb = sg.tile([128, 1], BF16, tag="ones_cb")
            eps_t = sg.tile([1, 1], F32, tag="eps_t")
            tri = sg.tile([128, 128], BF16, tag="tri")
            qhat = [
                [sg.tile([128, TC], BF16, tag=f"qh{h}_{j}", name=f"qh{h}_{j}")
                 for j in range(NTC)]
                for h in range(NH_L)
            ]
            khat = [sg.tile([128, TC], BF16, tag=f"kh{j}", name=f"kh{j}")
                    for j in range(NTC)]
            vhat = [sg.tile([128, TC], BF16, tag=f"vh{j}", name=f"vh{j}")
                    for j in range(NTC)]
            vtok = [sg.tile([128, TC], BF16, tag=f"vt{j}", name=f"vt{j}")
                    for j in range(NTC)]
            aout = [
                [sg.tile([128, TC], BF16, tag=f"ao{h}_{j}", name=f"ao{h}_{j}")
                 for j in range(NTC)]
                for h in range(NH_L)
            ]
            vhat2 = [sg.tile([128, TC], BF16, tag=f"vh2_{j}", name=f"vh2_{j}")
                     for j in range(NTC)]

            xts = {}

            def prefetch_x(j, g, split=1):
                """One DMA for D-chunks 4g..4g+3 of token chunk j."""
                tl = st.tile([128, 4 * TC], BF16, tag="xt", bufs=8,
                             name=f"xt{j}_{g}")
                xts[(j, g)] = tl
                nch = 4 // split
                for s in range(split):
                    src = xT[g * 512 + s * nch * 128:
                             g * 512 + (s + 1) * nch * 128,
                             j * TC:(j + 1) * TC]
                    if nch == 1:
                        nc.sync.dma_start(
                            out=tl[:, s * TC:(s + 1) * TC], in_=src)
                    else:
                        nc.sync.dma_start(
                            out=tl[:, s * nch * TC:(s + 1) * nch * TC]
                            .rearrange("p (c t) -> p c t", c=nch),
                            in_=src.rearrange("(c p) t -> p c t", p=128),
                        )

            def xt_slice(j, i):
                return xts[(j, i // 4)][:, (i % 4) * TC:(i % 4 + 1) * TC]

            # cs rides the ACT queue so w/x start immediately on SP
            nc.scalar.dma_start(out=cos_sb, in_=csP[0:RD, :])
            nc.scalar.dma_start(out=sin_sb, in_=csP[RD:128, :])
            # interleave w/x loads so QKV(0) can start after the first pair
            for g in range(4):
                prefetch_x(0, g, split=(2 if g == 0 else 1))
                for i in range(4 * g, 4 * g + 4):
                    nc.sync.dma_start(out=w_t[i],
                                      in_=wT[i * 128:(i + 1) * 128, :])

            # constants generated while the startup DMAs stream
            make_identity(nc, ident)
            nc.vector.memset(ones_cb, 1.0)
            nc.vector.memset(eps_t, EPS)
            # dummy exp: pulls the Exp act-table load into the startup
            # DMA wait instead of the first attention tile
            warm = st.tile([1, 1], F32, tag="warm", bufs=1)
            nc.scalar.activation(warm, eps_t, ACT.Exp, scale=1.0)
            nc.gpsimd.memset(tri, 1.0)
            # tri[k, q] = 1 where q >= k else 0 (diagonal 128x128 block)
            nc.gpsimd.affine_select(
                out=tri, in_=tri,
                compare_op=mybir.AluOpType.is_ge,
                fill=0.0,
                base=0,
                pattern=[[1, 128]],
                channel_multiplier=-1,
            )

            def emit_out_proj(jm1, mid_cb=None):
                """Out-projection for token chunk jm1 (PE filler work)."""
                for tl in range(4):
                    tt = jm1 * 4 + tl
                    if tl == 0 and mid_cb is not None:
                        mid_cb()
                    osb = st.tile([128, D], BF16, tag="osb", bufs=2,
                                  name=f"osb{tt}")
                    # final tile of the run: store per-m so the drain only
                    # waits on the last 512-column transfer
                    last = (jm1 == NTC - 1 and tl == 3)
                    for m in range(4):
                        ps_o = ps.tile([128, TC], F32, tag="B", bufs=3)
                        for h2 in range(NH_L):
                            nc.tensor.matmul(
                                ps_o,
                                aout[h2][jm1][:, tl * 128:(tl + 1) * 128],
                                wo_sb[h2][:, m * TC:(m + 1) * TC],
                                start=(h2 == 0), stop=(h2 == NH_L - 1),
                            )
                        if m % 2 == 0 or tl == 3:
                            nc.vector.tensor_copy(
                                osb[:, m * TC:(m + 1) * TC], ps_o)
                        else:
                            nc.scalar.copy(
                                osb[:, m * TC:(m + 1) * TC], ps_o)
                        if last:
                            nc.sync.dma_start(
                                out=outp[tt * 128:(tt + 1) * 128,
                                         m * TC:(m + 1) * TC],
                                in_=osb[:, m * TC:(m + 1) * TC])
                    if not last:
                        nc.sync.dma_start(
                            out=outp[tt * 128:(tt + 1) * 128, :], in_=osb)

            for j in range(NTC):
                js = slice(j * TC, (j + 1) * TC)
                # -------------- QKV pass A: q0, k, v + rs ------------------
                # k/v in pass A so their rope / v-scale / transpose chains
                # on DVE+Pool resolve while pass B runs on PE
                ps_rs = ps.tile([1, TC], F32, tag="S", bufs=2)
                psA = [ps.tile([128, TC], F32, tag="Q", bufs=3,
                               name=f"psA{j}_{e}") for e in range(3)]
                def emit_rs(i):
                    x2 = st.tile([128, TC], BF16, tag="x2", bufs=2)
                    nc.vector.tensor_mul(x2, xt_slice(j, i), xt_slice(j, i))
                    nc.tensor.matmul(ps_rs, ones_cb, x2,
                                     start=(i == 0), stop=(i == DC - 1))

                # rs matmuls shifted 6 behind qkv so chunk j's first PE inst
                # doesn't stall on the S-bank freed by attention(j-1) tail
                for i in range(DC):
                    xt_ji = xt_slice(j, i)
                    for ei, e in enumerate((0, 4, 5)):   # q0, k, v
                        nc.tensor.matmul(
                            psA[ei], w_t[i][:, e * 128:(e + 1) * 128], xt_ji,
                            start=(i == 0), stop=(i == DC - 1),
                        )
                    if i >= 6:
                        emit_rs(i - 6)
                for i in range(DC - 6, DC):
                    emit_rs(i)
                # rs = 1/sqrt(mean(x^2)+eps), broadcast across partitions
                sq = st.tile([1, TC], F32, tag="sq", bufs=1)
                nc.scalar.activation(sq, ps_rs, ACT.Sqrt, scale=1.0 / D,
                                     bias=eps_t)
                rs_t = st.tile([1, TC], F32, tag="rs_t", bufs=1)
                nc.vector.reciprocal(rs_t, sq)
                rsb = st.tile([128, TC], F32, tag="rsb", bufs=2)
                nc.gpsimd.partition_broadcast(rsb, rs_t)
                # fold rs into rope tables (in place, this chunk's columns)
                nc.vector.tensor_mul(cos_sb[:, js], cos_sb[:, js], rsb[0:RD])
                nc.vector.tensor_mul(sin_sb[:, js], sin_sb[:, js], rsb[0:RD])

                def rope(th, fast=False):
                    # fast=True keeps the whole chain on DVE: used for the
                    # tensors gating the first score matmuls (k, q0)
                    dt = th.dtype
                    t2s = st.tile([RD, TC], dt, tag="t2s", bufs=6)
                    t1 = st.tile([RD, TC], dt, tag="t1", bufs=3)
                    qeng = nc.scalar if fast else nc.sync
                    qeng.dma_start(out=t2s[0:32], in_=th[32:64])
                    qeng.dma_start(out=t2s[32:64], in_=th[0:32])
                    mul2 = nc.vector if fast else nc.gpsimd
                    mul2.tensor_mul(t2s, t2s, sin_sb[:, js])
                    nc.vector.tensor_mul(t1, th[0:RD], cos_sb[:, js])
                    nc.vector.tensor_add(th[0:RD], t1, t2s)
                    mul2.tensor_mul(th[RD:128], th[RD:128], rsb[RD:128])

                # evac pass A; k/q0 rope + vhat chains run under pass B
                nc.scalar.copy(khat[j], psA[1])
                nc.scalar.copy(vhat[j], psA[2])
                nc.vector.tensor_mul(vhat[j], vhat[j], rsb)
                nc.scalar.copy(qhat[0][j], psA[0])
                rope(khat[j], fast=(j == 0))
                rope(qhat[0][j], fast=(j == 0))

                # ---------------- QKV pass B: q1,q2,q3 ---------------------
                psB = [ps.tile([128, TC], F32, tag="Q", bufs=3,
                               name=f"psB{j}_{e}") for e in range(3)]
                for i in range(DC):
                    for e in range(3):
                        nc.tensor.matmul(
                            psB[e],
                            w_t[i][:, (1 + e) * 128:(2 + e) * 128],
                            xt_slice(j, i),
                            start=(i == 0), stop=(i == DC - 1),
                        )
                for h in range(1, 4):
                    nc.scalar.copy(qhat[h][j], psB[h - 1])
                    rope(qhat[h][j])

                # XSA v/|v|^2 term: only the head epilogues need it
                vsq = st.tile([128, TC], BF16, tag="vsq", bufs=1)
                nc.gpsimd.tensor_mul(vsq, vhat[j], vhat[j])
                vns_b = st.tile([128, TC], F32, tag="vns_b", bufs=1)
                nc.gpsimd.partition_all_reduce(
                    vns_b, vsq, channels=128,
                    reduce_op=bass.bass_isa.ReduceOp.add,
                )
                rvnsb = st.tile([128, TC], F32, tag="rvnsb", bufs=1)
                nc.vector.reciprocal(rvnsb, vns_b)
                nc.vector.tensor_mul(vhat2[j], vhat[j], rvnsb)

                def emit_vt():
                    ps_vt = ps.tile([128, TC], BF16, tag="B", bufs=3)
                    for kk in range(TC // 128):
                        nc.tensor.transpose(
                            ps_vt[:, kk * 128:(kk + 1) * 128],
                            vhat[j][:, kk * 128:(kk + 1) * 128],
                            ident,
                        )
                    if j == 0:
                        nc.vector.tensor_copy(vtok[j], ps_vt)
                    else:
                        nc.scalar.copy(vtok[j], ps_vt)

                # ---------------- attention for this q chunk ---------------
                nkt = 4 * (j + 1)
                ps_pv = {}
                ps_sum = {}
                pts = {}
                prefilled = set()

                def emit_sc(h, kt):
                    # diagonal blocks: queries < kt*128 are masked anyway,
                    # so compute only columns [lo:] (partial-width matmul)
                    lo = (kt - 4 * j) * 128 if kt >= 4 * j else 0
                    ps_sc = ps.tile([128, TC], F32, tag="B", bufs=3)
                    nc.tensor.matmul(
                        ps_sc[:, lo:],
                        khat[kt // 4][:, (kt % 4) * 128:(kt % 4 + 1) * 128],
                        qhat[h][j][:, lo:],
                        start=True, stop=True,
                    )
                    pT = st.tile([128, TC], BF16, tag="pT", bufs=6)
                    nc.scalar.activation(pT[:, lo:], ps_sc[:, lo:],
                                         ACT.Exp, scale=S128)
                    if kt >= 4 * j:  # causal triangle on first 128 cols
                        nc.vector.tensor_mul(
                            pT[:, lo:lo + 128], pT[:, lo:lo + 128], tri)
                    pts[(h, kt)] = (pT, lo)

                def emit_acc(h, kt):
                    pT, lo = pts.pop((h, kt))
                    nc.tensor.matmul(
                        ps_pv[h][:, lo:],
                        vtok[kt // 4][:, (kt % 4) * 128:(kt % 4 + 1) * 128],
                        pT[:, lo:],
                        start=(kt == 0), stop=(kt == nkt - 1),
                    )
                    nc.tensor.matmul(
                        ps_sum[h][:, lo:], ones_cb, pT[:, lo:],
                        start=(kt == 0), stop=(kt == nkt - 1),
                    )

                def emit_epilogue(h):
                    # normalization + XSA correction:
                    # aout = (pv - vhat2 * allreduce(pv*vhat)) / rowsum
                    # tu chain first: it is the longer serial path
                    tu = st.tile([128, TC], F32R, tag="tu", bufs=2)
                    nc.vector.tensor_mul(tu, ps_pv[h], vhat[j])
                    dotb = st.tile([128, TC], F32, tag="dotb", bufs=3)
                    nc.gpsimd.partition_all_reduce(
                        dotb, tu, channels=128,
                        reduce_op=bass.bass_isa.ReduceOp.add,
                    )
                    inv = st.tile([1, TC], F32, tag="inv", bufs=2)
                    nc.vector.reciprocal(inv, ps_sum[h])
                    ibc = st.tile([128, TC], F32, tag="bc", bufs=3, name="ibc")
                    nc.gpsimd.partition_broadcast(ibc, inv)
                    m2 = st.tile([128, TC], F32, tag="m2", bufs=3)
                    # j=0: kt loops are too thin to drain an all-DVE chain
                    eng2 = nc.gpsimd if j == 0 else nc.vector
                    eng2.tensor_mul(m2, vhat2[j], dotb)
                    s_ = st.tile([128, TC], F32, tag="s_", bufs=3)
                    nc.vector.tensor_sub(s_, ps_pv[h], m2)
                    eng2.tensor_mul(aout[h][j], s_, ibc)

                def prefill(h, depth):
                    ps_pv[h] = ps.tile([128, TC], F32, tag="Q", bufs=3,
                                       name=f"pv{j}_{h}")
                    ps_sum[h] = ps.tile([1, TC], F32, tag="S", bufs=2,
                                        name=f"sum{j}_{h}")
                    for kt in range(min(depth, nkt)):
                        emit_sc(h, kt)
                    prefilled.add(h)

                # -------- out-projection for the previous chunk ------------
                if j > 0:
                    # x prefetch first: no deps, so it issues on SP before
                    # the osb stores (whose sem-waits would block the queue)
                    if j + 1 < NTC:
                        for g in range(4):
                            prefetch_x(j + 1, g)
                    emit_out_proj(j - 1, mid_cb=emit_vt)
                else:
                    # fill the pre-transpose bubble with h0's first scores
                    prefill(0, 3)
                    emit_vt()
                    if NTC > 1:
                        for g in range(4):
                            prefetch_x(1, g)
                        # wo loads behind the x prefetch; needed from j=1
                        for h in range(NH_L):
                            nc.sync.dma_start(
                                out=wo_sb[h],
                                in_=woT[h * 128:(h + 1) * 128, :]
                            )

                for h in range(NH_L):
                    if h not in prefilled:
                        prefill(h, 3)
                    for kt in range(nkt):
                        if kt + 3 < nkt:
                            emit_sc(h, kt + 3)
                        emit_acc(h, kt)
                    emit_epilogue(h)

            # tail: out-projection of the last chunk
            emit_out_proj(NTC - 1)

    nc.compile()
    return nc


def _host_inputs(x, cos, sin, w_norm, wq, wk, wv, wo):
    """Build the 8 per-core input maps (host-side layout prep only)."""
    wn = w_norm.astype(np.float32)
    cosT = cos.T.astype(np.float32)                                # [64, T]
    sinT = sin.T.astype(np.float32)
    sinS = np.concatenate([-sinT[:32], sinT[32:]], axis=0)         # [64, T]
    cs = np.ascontiguousarray(np.concatenate([cosT, sinS], axis=0))  # [128, T]
    xTs = [np.ascontiguousarray(x[b].T).astype(ml_dtypes.bfloat16)
           for b in range(B)]
    in_maps = []
    for c in range(8):
        b, g = divmod(c, 4)
        wq_s = wq[g * NH_L * HD:(g + 1) * NH_L * HD] * wn[None, :]
        wk_s = wk[g * HD:(g + 1) * HD] * wn[None, :]
        wv_s = wv[g * HD:(g + 1) * HD] * wn[None, :]
        wqkvT = np.ascontiguousarray(
            np.concatenate([wq_s, wk_s, wv_s], axis=0).T
        ).astype(ml_dtypes.bfloat16)                               # [D, 768]
        woT_s = np.ascontiguousarray(
            wo[:, g * NH_L * HD:(g + 1) * NH_L * HD].T
        ).astype(ml_dtypes.bfloat16)                               # [512, D]
        in_maps.append({
            "xT": xTs[b],
            "wqkvT": wqkvT,
            "woT": woT_s,
            "cs": cs,
        })
    return in_maps


def kernel(x, cos, sin, w_norm, wq, wk, wv, wo, rope_dims=64, use_xsa=1,
           **_unused):
    if "nc" not in _CACHE:
        _CACHE["nc"] = _build_nc()
    nc = _CACHE["nc"]
    in_maps = _host_inputs(
        np.asarray(x), np.asarray(cos), np.asarray(sin), np.asarray(w_norm),
        np.asarray(wq), np.asarray(wk), np.asarray(wv), np.asarray(wo),
    )
    res_obj = run_bass_kernel_spmd(nc, in_maps, list(range(8)))
    _CACHE["last"] = res_obj
    res = res_obj.results
    out = np.zeros((B, T, D), dtype=np.float32)
    for c in range(8):
        b = c // 4
        out[b] += np.asarray(res[c]["out"], dtype=np.float32)
    return out
